# revision 47
# baseline (speedup 1.0000x reference)
"""Trainium2 Bass kernel: dense transformer block (B=2, T=2048, C=1024, H=16, HD=64).

Sharding over 8 NeuronCores: 2 batch groups (data parallel over B) x 4-way
tensor parallel within each group. Per group of 4 cores:
  - attention: heads split 4 ways (4 heads/core); per-core per-q-chunk
    attention outputs (bf16, feature-major) are exchanged with an AllToAll
    so each core ends up with the full 1024-dim attention for its own
    128-token block of every 512-token q-chunk; the output projection then
    runs locally against the full Wp.
  - FFN: token-split (512 tokens/core), full W1/W2 streamed per core in bf16.
Each core returns its 512-token shard of the final output; the host
reassembles the full [2, 2048, 1024] tensor.

Emission is pipelined per q-chunk (LN -> QKV -> scores/exp/AV -> normalize ->
AllToAll, with the next chunk's LN/QKV and the previous chunk's projection +
LN2 interleaved) so the tensor/scalar/vector queues always hold ready work
and the collectives overlap attention compute.
"""

import os
import sys

if "/opt/trn_rl_repo" not in sys.path:
    sys.path.insert(0, "/opt/trn_rl_repo")

import contextlib
import math

import ml_dtypes
import numpy as np

import concourse.bass as bass
import concourse.mybir as mybir
import concourse.tile as tile
from concourse import bacc
from concourse.bass_utils import run_bass_kernel_spmd
from concourse.masks import make_identity

# bass_utils' trace path imports antenv.axon_hooks, absent in this container.
# Register a graceful shim (and wire the boot-provided ctypes NTFF hook if
# available) so BASS_TRACE=1 profiles instead of crashing.
try:
    from antenv import axon_hooks as _ah  # noqa: F401
except ImportError:
    import types as _types

    _shim = _types.ModuleType("antenv.axon_hooks")
    _shim._hook = None
    _shim.set_axon_ntff_profile_hook = lambda h: setattr(_shim, "_hook", h)
    _shim.get_axon_ntff_profile_hook = lambda: _shim._hook
    sys.modules["antenv.axon_hooks"] = _shim
    try:
        if "/root/.axon_site" not in sys.path:
            sys.path.insert(0, "/root/.axon_site")
        from trn_agent_boot.trn_boot import _ntff_profile_via_ctypes

        _shim.set_axon_ntff_profile_hook(
            _ntff_profile_via_ctypes("/opt/axon/libaxon_pjrt.so")
        )
    except Exception:
        pass

AF = mybir.ActivationFunctionType
ALU = mybir.AluOpType
FP32 = mybir.dt.float32
BF16 = mybir.dt.bfloat16

P = 128
QCH = 512  # query chunk (free dim of S^T matmuls)
KG = 2  # k-tiles batched per exp() call


def build_block(T=2048, C=1024, NHL=4, F=4096, GC=4, eps=1e-5, n_cores=8):
    """Emit the per-core SPMD program. NHL = local heads (64-dim each)."""
    HD = 64
    DL = NHL * HD  # local head-dim total (256)
    NPAIR = NHL // 2
    NT = T // P  # token tiles (16)
    NCc = C // P  # channel tiles (8)
    NQC = T // QCH  # query chunks (4)
    KPC = QCH // P  # k-tiles / token tiles per chunk (4)
    TSH = T // GC  # token shard (512)
    NST = TSH // P  # shard token tiles (4)
    NHT = F // P  # FFN hidden tiles (32)
    scale = 1.0 / math.sqrt(HD)

    groups = [list(range(g * GC, (g + 1) * GC)) for g in range(n_cores // GC)]

    nc = bacc.Bacc(
        "TRN2", target_bir_lowering=False, num_devices=n_cores, debug=False
    )

    # ---- I/O ----
    x_full = nc.dram_tensor("x_full", [T, C], FP32, kind="ExternalInput")
    x_shard = nc.dram_tensor("x_shard", [TSH, C], FP32, kind="ExternalInput")
    wq_d = nc.dram_tensor("wq", [C, DL], BF16, kind="ExternalInput")
    wk_d = nc.dram_tensor("wk", [C, DL], BF16, kind="ExternalInput")
    wv_d = nc.dram_tensor("wv", [C, DL], BF16, kind="ExternalInput")
    wp_d = nc.dram_tensor("wp", [DL, C], BF16, kind="ExternalInput")
    w1_d = nc.dram_tensor("w1", [C, F], BF16, kind="ExternalInput")
    w2_d = nc.dram_tensor("w2", [F, C], BF16, kind="ExternalInput")
    b1_d = nc.dram_tensor("b1r", [P, NHT], FP32, kind="ExternalInput")
    bp_d = nc.dram_tensor("bp", [C], BF16, kind="ExternalInput")
    b2_d = nc.dram_tensor("b2", [C], BF16, kind="ExternalInput")
    g1_d = nc.dram_tensor("g1r", [P, NCc], FP32, kind="ExternalInput")
    be1_d = nc.dram_tensor("be1r", [P, NCc], FP32, kind="ExternalInput")
    g2_d = nc.dram_tensor("g2r", [P, NCc], FP32, kind="ExternalInput")
    be2_d = nc.dram_tensor("be2r", [P, NCc], FP32, kind="ExternalInput")
    mask_d = nc.dram_tensor("maskr", [P, P], BF16, kind="ExternalInput")
    out_d = nc.dram_tensor("out", [TSH, C], FP32, kind="ExternalOutput")

    with tile.TileContext(nc) as tc, contextlib.ExitStack() as est:
        sing = est.enter_context(tc.tile_pool(name="sing", bufs=1))
        tok = est.enter_context(tc.tile_pool(name="tok", bufs=2))
        tokE = est.enter_context(tc.tile_pool(name="tokE", bufs=3))
        xtp = est.enter_context(tc.tile_pool(name="xtp", bufs=4))
        stat = est.enter_context(tc.tile_pool(name="stat", bufs=2))
        vp = est.enter_context(tc.tile_pool(name="vp", bufs=NT))
        exps = est.enter_context(tc.tile_pool(name="exps", bufs=3))
        attp = est.enter_context(tc.tile_pool(name="attp", bufs=2))
        atgp = est.enter_context(tc.tile_pool(name="atgp", bufs=2))
        hid = est.enter_context(tc.tile_pool(name="hid", bufs=4))
        w1p = est.enter_context(tc.tile_pool(name="w1p", bufs=2))
        w2p = est.enter_context(tc.tile_pool(name="w2p", bufs=3))
        obp = est.enter_context(tc.tile_pool(name="obp", bufs=2))
        psS = est.enter_context(tc.tile_pool(name="psS", bufs=2, space="PSUM"))
        psAV = est.enter_context(tc.tile_pool(name="psAV", bufs=2, space="PSUM"))
        psQ = est.enter_context(tc.tile_pool(name="psQ", bufs=2, space="PSUM"))
        dram = est.enter_context(tc.tile_pool(name="dram", bufs=1, space="DRAM"))

        # ---- singles ----
        ident_bf = sing.tile([P, P], BF16, tag="ident", name="ident")
        make_identity(nc, ident_bf)
        eps_t = sing.tile([P, 1], FP32, tag="eps", name="eps")
        nc.vector.memset(eps_t, eps)
        ones_t = sing.tile([1, HD], BF16, tag="ones", name="ones")
        nc.vector.memset(ones_t, 1.0)

        # first tokens first: chunk 0's x tiles load before the weight bulk
        x0_ts = []
        for it in range(KPC):
            x_t = xtp.tile([P, C], FP32, tag="xb", name="xt")
            nc.sync.dma_start(x_t, x_full[it * P : (it + 1) * P, :])
            x0_ts.append(x_t)

        wq_sb = sing.tile([P, NCc, DL], BF16, tag="wq", name="wq")
        nc.sync.dma_start(wq_sb, wq_d.rearrange("(n p) m -> p n m", p=P))
        wk_sb = sing.tile([P, NCc, DL], BF16, tag="wk", name="wk")
        nc.sync.dma_start(wk_sb, wk_d.rearrange("(n p) m -> p n m", p=P))
        wv_sb = sing.tile([P, NCc, DL], BF16, tag="wv", name="wv")
        nc.sync.dma_start(wv_sb, wv_d.rearrange("(n p) m -> p n m", p=P))
        wp_sb = sing.tile([P, DL // P, C], BF16, tag="wp", name="wp")
        nc.sync.dma_start(wp_sb, wp_d.rearrange("(n p) m -> p n m", p=P))
        mask_sb = sing.tile([P, P], BF16, tag="mask", name="mask")
        nc.sync.dma_start(mask_sb, mask_d[:])
        b1_sb = sing.tile([P, NHT], FP32, tag="b1", name="b1")
        nc.sync.dma_start(b1_sb, b1_d[:])
        g1_sb = sing.tile([P, NCc], FP32, tag="g1", name="g1")
        nc.sync.dma_start(g1_sb, g1_d[:])
        be1_sb = sing.tile([P, NCc], FP32, tag="be1", name="be1")
        nc.sync.dma_start(be1_sb, be1_d[:])
        g2_sb = sing.tile([P, NCc], FP32, tag="g2", name="g2")
        nc.sync.dma_start(g2_sb, g2_d[:])
        be2_sb = sing.tile([P, NCc], FP32, tag="be2", name="be2")
        nc.sync.dma_start(be2_sb, be2_d[:])
        bp_bc = sing.tile([P, C], BF16, tag="bpbc", name="bpbc")
        bp_ap = bp_d[:]
        nc.sync.dma_start(
            bp_bc,
            bass.AP(tensor=bp_ap.tensor, offset=bp_ap.offset, ap=[[0, P]] + list(bp_ap.ap)),
        )
        b2_bc = sing.tile([P, C], BF16, tag="b2bc", name="b2bc")
        b2_ap = b2_d[:]
        nc.sync.dma_start(
            b2_bc,
            bass.AP(tensor=b2_ap.tensor, offset=b2_ap.offset, ap=[[0, P]] + list(b2_ap.ap)),
        )

        # persistent feature-major tiles
        hT = [
            sing.tile([P, T], BF16, tag=f"hT{ic}", name=f"hT{ic}") for ic in range(NCc)
        ]
        QT = [
            sing.tile([P, T], BF16, tag=f"QT{pr}", name=f"QT{pr}")
            for pr in range(NPAIR)
        ]
        KT = [
            sing.tile([P, T], BF16, tag=f"KT{pr}", name=f"KT{pr}")
            for pr in range(NPAIR)
        ]
        V4 = [None] * NT
        h2T = sing.tile([P, NCc, TSH], BF16, tag="h2T", name="h2T")
        x2pb = [
            sing.tile([P, C], FP32, tag=f"x2pb{st}", name=f"x2pb{st}")
            for st in range(NST)
        ]

        rs_in = [
            dram.tile([QCH, C], BF16, tag=f"rsi{k}", name=f"rsi{k}") for k in range(NQC)
        ]
        rs_out = [
            dram.tile([QCH // GC, C], BF16, tag=f"rso{k}", name=f"rso{k}")
            for k in range(NQC)
        ]

        # ---- LayerNorm stats helpers ----
        # Stats for a batch of token-major [P, C] tiles; ONE batched sqrt so
        # the scalar engine swaps its activation table (EXP<->SQRT) once per
        # chunk instead of once per tile.
        nsub = C // 512

        def ln_stats_batch(x_ts, tagp):
            n = len(x_ts)
            mv = stat.tile([P, n, 2], FP32, tag=f"{tagp}_mv", name=f"{tagp}_mv")
            nm = stat.tile([P, n], FP32, tag=f"{tagp}_nm", name=f"{tagp}_nm")
            for i, x_t in enumerate(x_ts):
                stats = stat.tile(
                    [P, nsub, 6], FP32, tag=f"{tagp}_st", name=f"{tagp}_st"
                )
                xr = x_t.rearrange("p (n f) -> p n f", n=nsub)
                for s in range(nsub):
                    nc.vector.bn_stats(out=stats[:, s, :], in_=xr[:, s, :])
                nc.vector.bn_aggr(out=mv[:, i, :], in_=stats)
            # mv[:,:,0]=mean  mv[:,:,1]=var -> rstd in place (batched sqrt)
            nc.scalar.activation(
                out=mv[:, :, 1], in_=mv[:, :, 1], func=AF.Sqrt, bias=eps_t, scale=1.0
            )
            nc.vector.reciprocal(out=mv[:, :, 1], in_=mv[:, :, 1])
            for i in range(n):
                nc.vector.tensor_scalar(
                    out=nm[:, i : i + 1],
                    in0=mv[:, i, 0:1],
                    scalar1=-1.0,
                    scalar2=mv[:, i, 1:2],
                    op0=ALU.mult,
                    op1=ALU.mult,
                )
            return mv, nm

        def copy_scaled(dst, src, g_ap, b_ap, on_vector):
            if on_vector:
                nc.vector.tensor_scalar(
                    out=dst, in0=src, scalar1=g_ap, scalar2=b_ap,
                    op0=ALU.mult, op1=ALU.add,
                )
            else:
                nc.scalar.activation(
                    out=dst, in_=src, func=AF.Identity, bias=b_ap, scale=g_ap
                )

        # ================= per-chunk emitters =================

        def emit_A(ch):
            """LN1 + transpose + V for token tiles of chunk ch.

            Transposes run in bf16 (fp32 PE transposes are 2-pass, ~4x the
            cost), and the V contraction interleaves token-tile pairs so
            consecutive matmuls accumulate into different PSUM banks.
            """
            its = list(range(ch * KPC, (ch + 1) * KPC))
            x_ts = {}
            for it in its:
                if ch == 0:
                    x_ts[it] = x0_ts[it]  # preloaded before the weight bulk
                    continue
                x_t = xtp.tile([P, C], FP32, tag="xb", name="xt")
                nc.sync.dma_start(x_t, x_full[it * P : (it + 1) * P, :])
                x_ts[it] = x_t
            mv, nm = ln_stats_batch([x_ts[it] for it in its], "ln1")
            for i, it in enumerate(its):
                h_t = tok.tile([P, C], BF16, tag="tb", name="ht")
                # h = (x - mu) * rstd  == x*rstd + (-mu*rstd)
                nc.vector.tensor_scalar(
                    out=h_t, in0=x_ts[it], scalar1=mv[:, i, 1:2], scalar2=nm[:, i : i + 1],
                    op0=ALU.mult, op1=ALU.add,
                )
                for ic in range(NCc):
                    ps = psQ.tile([P, P], BF16, tag="psQ", name="psQ")
                    nc.tensor.transpose(ps, h_t[:, ic * P : (ic + 1) * P], ident_bf)
                    copy_scaled(
                        hT[ic][:, it * P : (it + 1) * P],
                        ps,
                        g1_sb[:, ic : ic + 1],
                        be1_sb[:, ic : ic + 1],
                        on_vector=((it + ic) % 2 == 0),
                    )
            # V (token-major [P, DL]), token tiles pairwise interleaved
            for it0 in its[::2]:
                pair = (it0, it0 + 1)
                pss = {}
                for it in pair:
                    pss[it] = psQ.tile([P, DL], FP32, tag="psQ", name="psQ")
                for ic in range(NCc):
                    for it in pair:
                        nc.tensor.matmul(
                            pss[it],
                            lhsT=hT[ic][:, it * P : (it + 1) * P],
                            rhs=wv_sb[:, ic, :],
                            start=(ic == 0),
                            stop=(ic == NCc - 1),
                        )
                for it in pair:
                    # [P, head, 65]: col 64 = ones so the AV matmul also
                    # produces the softmax rowsum (no separate rowsum matmul)
                    v_t = vp.tile([P, NHL, HD + 1], BF16, tag="v", name="v")
                    if it % 2 == 0:
                        nc.vector.tensor_copy(
                            v_t[:, :, 0:HD],
                            pss[it].rearrange("p (h d) -> p h d", h=NHL),
                        )
                    else:
                        nc.scalar.copy(
                            v_t[:, :, 0:HD],
                            pss[it].rearrange("p (h d) -> p h d", h=NHL),
                        )
                    nc.vector.memset(v_t[:, :, HD], 1.0)
                    V4[it] = v_t

        def emit_B(ch):
            """Q^T / K^T (feature-major, head-pair stacked) for chunk ch.

            Q and K contractions interleave so consecutive matmuls hit
            different PSUM banks (keeps the PE fill/drain pipelined).
            """
            for pr in range(NPAIR):
                ps_q = psQ.tile([P, QCH], FP32, tag="psQ", name="psQ")
                ps_k = psQ.tile([P, QCH], FP32, tag="psQ", name="psQ")
                for ic in range(NCc):
                    for ps, w_sb in ((ps_q, wq_sb), (ps_k, wk_sb)):
                        nc.tensor.matmul(
                            ps,
                            lhsT=w_sb[:, ic, pr * P : (pr + 1) * P],
                            rhs=hT[ic][:, ch * QCH : (ch + 1) * QCH],
                            start=(ic == 0),
                            stop=(ic == NCc - 1),
                        )
                for qi, (dst_tiles, ps) in enumerate(((QT, ps_q), (KT, ps_k))):
                    dst = dst_tiles[pr][:, ch * QCH : (ch + 1) * QCH]
                    if (pr + qi) % 2 == 0:
                        nc.vector.tensor_copy(dst, ps)
                    else:
                        nc.scalar.copy(dst, ps)

        def emit_C(ch):
            """scores + softmax-exp + AV + normalize + proj + ReduceScatter."""
            n_kt = (ch + 1) * KPC
            attT = []
            for pr in range(NPAIR):
                att_ps = [
                    psAV.tile([P, QCH], FP32, tag="psAV", name="psAV")
                    for _ in range(2)
                ]
                # scores tile per k-tile holds BOTH heads ((pos, q) layout):
                # ONE exp instruction unlocks both heads' AVs, and the psS
                # double-buffer gives a true depth-2 pipeline (exp of k-tile
                # n overlaps the AVs of n-1 and scores of n+1).
                for kt in range(n_kt):
                    s_ps = psS.tile([P, 2, QCH], FP32, tag="psS", name="psS")
                    for pos in range(2):
                        nc.tensor.matmul(
                            s_ps[:, pos, :],
                            lhsT=KT[pr][
                                64 * pos : 64 * pos + 64, kt * P : (kt + 1) * P
                            ],
                            rhs=QT[pr][
                                64 * pos : 64 * pos + 64,
                                ch * QCH : (ch + 1) * QCH,
                            ],
                            start=True,
                            stop=True,
                            tile_position=(64 * pos, 0),
                        )
                    e_sb = exps.tile([P, 2, QCH], BF16, tag="e", name="e")
                    nc.scalar.activation(
                        out=e_sb, in_=s_ps, func=AF.Exp, scale=scale
                    )
                    jd = kt - ch * KPC
                    if 0 <= jd < KPC:
                        # cols < jd*P fully masked; [jd*P,(jd+1)*P)
                        # triangular; rest fully visible.
                        for pos in range(2):
                            if jd > 0:
                                nc.vector.memset(e_sb[:, pos, 0 : jd * P], 0.0)
                            nc.vector.tensor_mul(
                                e_sb[:, pos, jd * P : (jd + 1) * P],
                                e_sb[:, pos, jd * P : (jd + 1) * P],
                                mask_sb,
                            )
                    # AV (rowsum rides along as the 65th lhsT column);
                    # pos streams alternate PSUM banks
                    for pos in range(2):
                        lh = 2 * pr + pos
                        nc.tensor.matmul(
                            att_ps[pos][0:65, :],
                            lhsT=V4[kt][:, lh, :],
                            rhs=e_sb[:, pos, :],
                            start=(kt == 0),
                            stop=(kt == n_kt - 1),
                        )
                # normalize: reciprocal on the [1, QCH] rowsum row, broadcast
                # via outer-product matmul, multiply on vector.
                bc_ps = psQ.tile([P, QCH], FP32, tag="psQ", name="psQ")
                for pos in range(2):
                    rrec = stat.tile([1, QCH], BF16, tag=f"rr{pos}", name="rrec")
                    with nc.allow_low_precision(reason="1/rowsum as bf16 matmul rhs"):
                        nc.vector.reciprocal(out=rrec, in_=att_ps[pos][64:65, :])
                    nc.tensor.matmul(
                        bc_ps[64 * pos : 64 * pos + 64, :],
                        lhsT=ones_t,
                        rhs=rrec,
                        start=True,
                        stop=True,
                        tile_position=(0, 64 * pos),
                        skip_group_check=(pos == 1),
                    )
                rec_sb = exps.tile([P, QCH], BF16, tag="rsb", name="rsb")
                nc.vector.tensor_copy(rec_sb, bc_ps)
                at = attp.tile([P, QCH], BF16, tag="attT", name="attT")
                for pos in range(2):
                    rows = slice(64 * pos, 64 * pos + 64)
                    nc.vector.tensor_mul(
                        at[rows, :], att_ps[pos][0:64, :], rec_sb[rows, :]
                    )
                attT.append(at)
            # proj partials (local heads, all chunk tokens) -> bf16 -> rs_in;
            # the two output-column halves interleave (PSUM bank alternation)
            for tt in range(KPC):
                pjs = [psQ.tile([P, 512], FP32, tag="psQ", name="psQ") for _ in range(2)]
                for pr in range(NPAIR):
                    for oc in range(2):
                        nc.tensor.matmul(
                            pjs[oc],
                            lhsT=attT[pr][:, tt * P : (tt + 1) * P],
                            rhs=wp_sb[:, pr, oc * 512 : (oc + 1) * 512],
                            start=(pr == 0),
                            stop=(pr == NPAIR - 1),
                        )
                for oc in range(2):
                    pj_sb = exps.tile([P, 512], BF16, tag="pj", name="pjsb")
                    if (tt + oc) % 2 == 0:
                        nc.vector.tensor_copy(pj_sb, pjs[oc])
                    else:
                        nc.scalar.copy(pj_sb, pjs[oc])
                    nc.sync.dma_start(
                        rs_in[ch][tt * P : (tt + 1) * P, oc * 512 : (oc + 1) * 512],
                        pj_sb,
                    )
            nc.gpsimd.collective_compute(
                "ReduceScatter",
                ALU.add,
                replica_groups=groups,
                ins=[rs_in[ch][:].opt()],
                outs=[rs_out[ch][:].opt()],
            )

        def emit_P(ch):
            """residual for our 128-token block of chunk ch + LN2."""
            st = ch  # shard token tile == chunk index
            r_t = atgp.tile([P, C], BF16, tag="atg", name="rt")
            nc.sync.dma_start(r_t, rs_out[ch][:])
            xs_t = tokE.tile([P, C], FP32, tag="te", name="xst")
            nc.sync.dma_start(xs_t, x_shard[st * P : (st + 1) * P, :])
            x2 = x2pb[st]
            nc.vector.tensor_add(x2, r_t, xs_t)
            nc.gpsimd.tensor_add(x2, x2, bp_bc)
            mv, nm = ln_stats_batch([x2], "ln2")
            h2f = tokE.tile([P, C], BF16, tag="te", name="h2f")
            nc.vector.tensor_scalar(
                out=h2f, in0=x2, scalar1=mv[:, 0, 1:2], scalar2=nm[:, 0:1],
                op0=ALU.mult, op1=ALU.add,
            )
            for ic in range(NCc):
                ps = psQ.tile([P, P], BF16, tag="psQ", name="psQ")
                nc.tensor.transpose(ps, h2f[:, ic * P : (ic + 1) * P], ident_bf)
                copy_scaled(
                    h2T[:, ic, st * P : (st + 1) * P],
                    ps,
                    g2_sb[:, ic : ic + 1],
                    be2_sb[:, ic : ic + 1],
                    on_vector=((st + ic) % 2 == 0),
                )
            # after LN2 consumed x2, fold in b2 for the FFN residual
            nc.gpsimd.tensor_add(x2, x2, b2_bc)

        # ================= emission =================
        emit_A(0)
        emit_B(0)
        for ch in range(NQC):
            emit_C(ch)
            if ch + 1 < NQC:
                emit_A(ch + 1)
                emit_B(ch + 1)
        # all residual+LN2 blocks after the attention stream: P(0..2) fill
        # the RS3 tail window, and no P op can head-of-line-block the
        # engine queues mid-attention.
        for ch in range(NQC):
            emit_P(ch)

        # ================= FFN =================
        HPC = (2 * T) // TSH  # h-tiles per hidT chunk (8)
        n_hc = (NHT + HPC - 1) // HPC
        hidT = [
            hid.tile([P, 2 * T], BF16, tag="hid", name=f"hid{j}") for j in range(n_hc)
        ]

        def hid_slice(ht, t0, tlen):
            j, o = ht // HPC, ht % HPC
            return hidT[j][:, o * TSH + t0 : o * TSH + t0 + tlen]

        n_grp = (NST + 1) // 2
        grp0 = [
            psS.tile([P, min(2, NST) * 512], FP32, tag="psS", name="psS")
            for _ in range(n_grp)
        ]

        def grp_slice(grp, tt):
            return grp[tt // 2][:, (tt % 2) * 512 : (tt % 2 + 1) * 512]

        def emit_w2(ht, w2t, grp):
            for tt in range(NST):
                nc.tensor.matmul(
                    grp_slice(grp, tt),
                    lhsT=hid_slice(ht, tt * P, P),
                    rhs=w2t,
                    start=(ht == 0),
                    stop=(ht == NHT - 1),
                )

        pending = []  # W2 of a tile pair trails the next pair's hid matmuls
        # so the relu is never on the PE critical path.
        for h2 in range(NHT // 2):
            # hid matmuls for a pair of h-tiles, interleaved so consecutive
            # matmuls hit different PSUM banks
            hts = (2 * h2, 2 * h2 + 1)
            w1t = w1p.tile([P, NCc, 2 * P], BF16, tag="w1t", name="w1t")
            for ic in range(NCc):
                nc.sync.dma_start(
                    w1t[:, ic, :],
                    w1_d[ic * P : (ic + 1) * P, hts[0] * P : (hts[0] + 2) * P],
                )
            hps = {ht: psQ.tile([P, TSH], FP32, tag="psQ", name="psQ") for ht in hts}
            for ic in range(NCc):
                for ht in hts:
                    nc.tensor.matmul(
                        hps[ht],
                        lhsT=w1t[:, ic, (ht % 2) * P : (ht % 2 + 1) * P],
                        rhs=h2T[:, ic, :],
                        start=(ic == 0),
                        stop=(ic == NCc - 1),
                    )
            for ht in hts:
                nc.scalar.activation(
                    out=hid_slice(ht, 0, TSH),
                    in_=hps[ht],
                    func=AF.Relu,
                    bias=b1_sb[:, ht : ht + 1],
                    scale=1.0,
                )
            w2t = w2p.tile([P, 2, 512], BF16, tag="w2t", name="w2t")
            nc.sync.dma_start(
                w2t,
                w2_d[hts[0] * P : (hts[0] + 2) * P, 0:512].rearrange(
                    "(n p) m -> p n m", p=P
                ),
            )
            for ht0, w2t0 in pending:
                for k in range(2):
                    emit_w2(ht0 + k, w2t0[:, k, :], grp0)
            pending = [(hts[0], w2t)]
        for ht0, w2t0 in pending:
            for k in range(2):
                emit_w2(ht0 + k, w2t0[:, k, :], grp0)
        pending = []
        for tt in range(NST):
            ob = obp.tile([P, 512], FP32, tag="ob", name="ob")
            nc.vector.tensor_add(ob, grp_slice(grp0, tt), x2pb[tt][:, 0:512])
            nc.sync.dma_start(out_d[tt * P : (tt + 1) * P, 0:512], ob)
        grp1 = [
            psS.tile([P, min(2, NST) * 512], FP32, tag="psS", name="psS")
            for _ in range(n_grp)
        ]
        for h2 in range(NHT // 2):
            w2t = w2p.tile([P, 2, 512], BF16, tag="w2t", name="w2t")
            nc.sync.dma_start(
                w2t,
                w2_d[2 * h2 * P : (2 * h2 + 2) * P, 512:1024].rearrange(
                    "(n p) m -> p n m", p=P
                ),
            )
            for k in range(2):
                emit_w2(2 * h2 + k, w2t[:, k, :], grp1)
        for tt in range(NST):
            ob = obp.tile([P, 512], FP32, tag="ob", name="ob")
            nc.vector.tensor_add(ob, grp_slice(grp1, tt), x2pb[tt][:, 512:1024])
            nc.sync.dma_start(out_d[tt * P : (tt + 1) * P, 512:1024], ob)

    nc.finalize()
    return nc


# ------------------------- host side -------------------------

_CACHE = {}
LAST_RESULTS = None


def make_in_maps(inputs, T=2048, C=1024, H=16, F=4096, GC=4, n_cores=8):
    HD = 64
    NHL = H // GC
    DL = NHL * HD
    NHT = F // P
    NCc = C // P
    bf = ml_dtypes.bfloat16

    x = np.asarray(inputs["x"], np.float32)
    Wq = np.asarray(inputs["Wq"], np.float32)
    Wk = np.asarray(inputs["Wk"], np.float32)
    Wv = np.asarray(inputs["Wv"], np.float32)
    Wp = np.asarray(inputs["Wp"], np.float32)
    bp = np.asarray(inputs["bp"], np.float32)
    W1 = np.asarray(inputs["W1"], np.float32)
    b1 = np.asarray(inputs["b1"], np.float32)
    W2 = np.asarray(inputs["W2"], np.float32)
    b2 = np.asarray(inputs["b2"], np.float32)
    g1 = np.asarray(inputs["g1"], np.float32)
    be1 = np.asarray(inputs["beta1"], np.float32)
    g2 = np.asarray(inputs["g2"], np.float32)
    be2 = np.asarray(inputs["beta2"], np.float32)

    maskr = np.triu(np.ones((P, P), np.float32)).astype(bf)  # m[kr,qr]=kr<=qr
    b1r = np.ascontiguousarray(b1.reshape(NHT, P).T)
    g1r = np.ascontiguousarray(g1.reshape(NCc, P).T)
    be1r = np.ascontiguousarray(be1.reshape(NCc, P).T)
    g2r = np.ascontiguousarray(g2.reshape(NCc, P).T)
    be2r = np.ascontiguousarray(be2.reshape(NCc, P).T)
    w1b = W1.astype(bf)
    w2b = W2.astype(bf)

    NQC = T // QCH
    RPC = QCH // GC  # shard rows per q-chunk (128)

    def shard_rows(g):
        return np.concatenate(
            [np.arange(k * QCH + g * RPC, k * QCH + (g + 1) * RPC) for k in range(NQC)]
        )

    in_maps = []
    for c in range(n_cores):
        b, g = c // GC, c % GC
        hsl = slice(g * NHL, (g + 1) * NHL)
        in_maps.append(
            {
                "x_full": np.ascontiguousarray(x[b]),
                "x_shard": np.ascontiguousarray(x[b][shard_rows(g)]),
                "wq": np.ascontiguousarray(
                    Wq[hsl].transpose(1, 0, 2).reshape(C, DL)
                ).astype(bf),
                "wk": np.ascontiguousarray(
                    Wk[hsl].transpose(1, 0, 2).reshape(C, DL)
                ).astype(bf),
                "wv": np.ascontiguousarray(
                    Wv[hsl].transpose(1, 0, 2).reshape(C, DL)
                ).astype(bf),
                "wp": np.ascontiguousarray(Wp[g * DL : (g + 1) * DL]).astype(bf),
                "w1": w1b,
                "w2": w2b,
                "b1r": b1r,
                "bp": bp.astype(bf),
                "b2": b2.astype(bf),
                "g1r": g1r,
                "be1r": be1r,
                "g2r": g2r,
                "be2r": be2r,
                "maskr": maskr,
            }
        )
    return in_maps


def kernel(**inputs) -> np.ndarray:
    global LAST_RESULTS
    B, T, C = inputs["x"].shape
    H = inputs["Wq"].shape[0]
    F = inputs["W1"].shape[1]
    GC = 4
    n_cores = 8
    key = (T, C, H, F)
    if key not in _CACHE:
        _CACHE[key] = build_block(T=T, C=C, NHL=H // GC, F=F, GC=GC, n_cores=n_cores)
    nc = _CACHE[key]
    in_maps = make_in_maps(inputs, T=T, C=C, H=H, F=F, GC=GC, n_cores=n_cores)
    res = run_bass_kernel_spmd(nc, in_maps, core_ids=list(range(n_cores)))
    LAST_RESULTS = res
    out = np.empty((B, T, C), np.float32)
    NQC = T // QCH
    RPC = QCH // GC
    for c in range(n_cores):
        b, g = c // GC, c % GC
        sh = res.results[c]["out"]
        for k in range(NQC):
            out[b, k * QCH + g * RPC : k * QCH + (g + 1) * RPC] = sh[
                k * RPC : (k + 1) * RPC
            ]
    return out


# revision 48
# speedup vs baseline: 1.0332x; 1.0332x over previous
"""Trainium2 Bass kernel: dense transformer block (B=2, T=2048, C=1024, H=16, HD=64).

Sharding over 8 NeuronCores: 2 batch groups (data parallel over B) x 4-way
tensor parallel within each group. Per group of 4 cores:
  - attention: heads split 4 ways (4 heads/core); per-core per-q-chunk
    attention outputs (bf16, feature-major) are exchanged with an AllToAll
    so each core ends up with the full 1024-dim attention for its own
    128-token block of every 512-token q-chunk; the output projection then
    runs locally against the full Wp.
  - FFN: token-split (512 tokens/core), full W1/W2 streamed per core in bf16.
Each core returns its 512-token shard of the final output; the host
reassembles the full [2, 2048, 1024] tensor.

Emission is pipelined per q-chunk (LN -> QKV -> scores/exp/AV -> normalize ->
AllToAll, with the next chunk's LN/QKV and the previous chunk's projection +
LN2 interleaved) so the tensor/scalar/vector queues always hold ready work
and the collectives overlap attention compute.
"""

import os
import sys

if "/opt/trn_rl_repo" not in sys.path:
    sys.path.insert(0, "/opt/trn_rl_repo")

import contextlib
import math

import ml_dtypes
import numpy as np

import concourse.bass as bass
import concourse.mybir as mybir
import concourse.tile as tile
from concourse import bacc
from concourse.bass_utils import run_bass_kernel_spmd
from concourse.masks import make_identity

# bass_utils' trace path imports antenv.axon_hooks, absent in this container.
# Register a graceful shim (and wire the boot-provided ctypes NTFF hook if
# available) so BASS_TRACE=1 profiles instead of crashing.
try:
    from antenv import axon_hooks as _ah  # noqa: F401
except ImportError:
    import types as _types

    _shim = _types.ModuleType("antenv.axon_hooks")
    _shim._hook = None
    _shim.set_axon_ntff_profile_hook = lambda h: setattr(_shim, "_hook", h)
    _shim.get_axon_ntff_profile_hook = lambda: _shim._hook
    sys.modules["antenv.axon_hooks"] = _shim
    try:
        if "/root/.axon_site" not in sys.path:
            sys.path.insert(0, "/root/.axon_site")
        from trn_agent_boot.trn_boot import _ntff_profile_via_ctypes

        _shim.set_axon_ntff_profile_hook(
            _ntff_profile_via_ctypes("/opt/axon/libaxon_pjrt.so")
        )
    except Exception:
        pass

AF = mybir.ActivationFunctionType
ALU = mybir.AluOpType
FP32 = mybir.dt.float32
BF16 = mybir.dt.bfloat16

P = 128
QCH = 512  # query chunk (free dim of S^T matmuls)
KG = 2  # k-tiles batched per exp() call


def build_block(T=2048, C=1024, NHL=4, F=4096, GC=4, eps=1e-5, n_cores=8):
    """Emit the per-core SPMD program. NHL = local heads (64-dim each)."""
    HD = 64
    DL = NHL * HD  # local head-dim total (256)
    NPAIR = NHL // 2
    NT = T // P  # token tiles (16)
    NCc = C // P  # channel tiles (8)
    NQC = T // QCH  # query chunks (4)
    KPC = QCH // P  # k-tiles / token tiles per chunk (4)
    TSH = T // GC  # token shard (512)
    NST = TSH // P  # shard token tiles (4)
    NHT = F // P  # FFN hidden tiles (32)
    scale = 1.0 / math.sqrt(HD)

    groups = [list(range(g * GC, (g + 1) * GC)) for g in range(n_cores // GC)]

    nc = bacc.Bacc(
        "TRN2", target_bir_lowering=False, num_devices=n_cores, debug=False
    )

    # ---- I/O ----
    x_full = nc.dram_tensor("x_full", [T, C], FP32, kind="ExternalInput")
    x_shard = nc.dram_tensor("x_shard", [TSH, C], FP32, kind="ExternalInput")
    wq_d = nc.dram_tensor("wq", [C, DL], BF16, kind="ExternalInput")
    wk_d = nc.dram_tensor("wk", [C, DL], BF16, kind="ExternalInput")
    wv_d = nc.dram_tensor("wv", [C, DL], BF16, kind="ExternalInput")
    wp_d = nc.dram_tensor("wp", [DL, C], BF16, kind="ExternalInput")
    w1_d = nc.dram_tensor("w1", [C, F], BF16, kind="ExternalInput")
    w2_d = nc.dram_tensor("w2", [F, C], BF16, kind="ExternalInput")
    b1_d = nc.dram_tensor("b1r", [P, NHT], FP32, kind="ExternalInput")
    bp_d = nc.dram_tensor("bp", [C], BF16, kind="ExternalInput")
    b2_d = nc.dram_tensor("b2", [C], BF16, kind="ExternalInput")
    g1_d = nc.dram_tensor("g1r", [P, NCc], FP32, kind="ExternalInput")
    be1_d = nc.dram_tensor("be1r", [P, NCc], FP32, kind="ExternalInput")
    g2_d = nc.dram_tensor("g2r", [P, NCc], FP32, kind="ExternalInput")
    be2_d = nc.dram_tensor("be2r", [P, NCc], FP32, kind="ExternalInput")
    mask_d = nc.dram_tensor("maskr", [P, P], BF16, kind="ExternalInput")
    out_d = nc.dram_tensor("out", [TSH, C], FP32, kind="ExternalOutput")

    with tile.TileContext(nc) as tc, contextlib.ExitStack() as est:
        sing = est.enter_context(tc.tile_pool(name="sing", bufs=1))
        tok = est.enter_context(tc.tile_pool(name="tok", bufs=3))
        tokE = est.enter_context(tc.tile_pool(name="tokE", bufs=3))
        xtp = est.enter_context(tc.tile_pool(name="xtp", bufs=4))
        stat = est.enter_context(tc.tile_pool(name="stat", bufs=2))
        vp = est.enter_context(tc.tile_pool(name="vp", bufs=NT))
        exps = est.enter_context(tc.tile_pool(name="exps", bufs=3))
        attp = est.enter_context(tc.tile_pool(name="attp", bufs=2))
        atgp = est.enter_context(tc.tile_pool(name="atgp", bufs=2))
        hid = est.enter_context(tc.tile_pool(name="hid", bufs=4))
        w1p = est.enter_context(tc.tile_pool(name="w1p", bufs=2))
        w2p = est.enter_context(tc.tile_pool(name="w2p", bufs=2))
        obp = est.enter_context(tc.tile_pool(name="obp", bufs=2))
        psS = est.enter_context(tc.tile_pool(name="psS", bufs=2, space="PSUM"))
        psAV = est.enter_context(tc.tile_pool(name="psAV", bufs=2, space="PSUM"))
        psQ = est.enter_context(tc.tile_pool(name="psQ", bufs=2, space="PSUM"))
        dram = est.enter_context(tc.tile_pool(name="dram", bufs=1, space="DRAM"))

        # ---- singles ----
        ident_bf = sing.tile([P, P], BF16, tag="ident", name="ident")
        make_identity(nc, ident_bf)
        eps_t = sing.tile([P, 1], FP32, tag="eps", name="eps")
        nc.vector.memset(eps_t, eps)
        ones_t = sing.tile([1, HD], BF16, tag="ones", name="ones")
        nc.vector.memset(ones_t, 1.0)

        # first tokens first: chunk 0's x tiles load before the weight bulk
        x0_ts = []
        for it in range(KPC):
            x_t = xtp.tile([P, C], FP32, tag="xb", name="xt")
            nc.sync.dma_start(x_t, x_full[it * P : (it + 1) * P, :])
            x0_ts.append(x_t)

        wq_sb = sing.tile([P, NCc, DL], BF16, tag="wq", name="wq")
        nc.sync.dma_start(wq_sb, wq_d.rearrange("(n p) m -> p n m", p=P))
        wk_sb = sing.tile([P, NCc, DL], BF16, tag="wk", name="wk")
        nc.sync.dma_start(wk_sb, wk_d.rearrange("(n p) m -> p n m", p=P))
        wv_sb = sing.tile([P, NCc, DL], BF16, tag="wv", name="wv")
        nc.sync.dma_start(wv_sb, wv_d.rearrange("(n p) m -> p n m", p=P))
        wp_sb = sing.tile([P, DL // P, C], BF16, tag="wp", name="wp")
        nc.sync.dma_start(wp_sb, wp_d.rearrange("(n p) m -> p n m", p=P))
        mask_sb = sing.tile([P, P], BF16, tag="mask", name="mask")
        nc.sync.dma_start(mask_sb, mask_d[:])
        b1_sb = sing.tile([P, NHT], FP32, tag="b1", name="b1")
        nc.sync.dma_start(b1_sb, b1_d[:])
        g1_sb = sing.tile([P, NCc], FP32, tag="g1", name="g1")
        nc.sync.dma_start(g1_sb, g1_d[:])
        be1_sb = sing.tile([P, NCc], FP32, tag="be1", name="be1")
        nc.sync.dma_start(be1_sb, be1_d[:])
        g2_sb = sing.tile([P, NCc], FP32, tag="g2", name="g2")
        nc.sync.dma_start(g2_sb, g2_d[:])
        be2_sb = sing.tile([P, NCc], FP32, tag="be2", name="be2")
        nc.sync.dma_start(be2_sb, be2_d[:])
        bp_bc = sing.tile([P, C], BF16, tag="bpbc", name="bpbc")
        bp_ap = bp_d[:]
        nc.sync.dma_start(
            bp_bc,
            bass.AP(tensor=bp_ap.tensor, offset=bp_ap.offset, ap=[[0, P]] + list(bp_ap.ap)),
        )
        b2_bc = sing.tile([P, C], BF16, tag="b2bc", name="b2bc")
        b2_ap = b2_d[:]
        nc.sync.dma_start(
            b2_bc,
            bass.AP(tensor=b2_ap.tensor, offset=b2_ap.offset, ap=[[0, P]] + list(b2_ap.ap)),
        )

        # persistent feature-major tiles
        hT = [
            sing.tile([P, T], BF16, tag=f"hT{ic}", name=f"hT{ic}") for ic in range(NCc)
        ]
        QT = [
            sing.tile([P, T], BF16, tag=f"QT{pr}", name=f"QT{pr}")
            for pr in range(NPAIR)
        ]
        KT = [
            sing.tile([P, T], BF16, tag=f"KT{pr}", name=f"KT{pr}")
            for pr in range(NPAIR)
        ]
        V4 = [None] * NT
        h2T = sing.tile([P, NCc, TSH], BF16, tag="h2T", name="h2T")
        x2pb = [
            sing.tile([P, C], FP32, tag=f"x2pb{st}", name=f"x2pb{st}")
            for st in range(NST)
        ]

        rs_in = [
            dram.tile([QCH, C], BF16, tag=f"rsi{k}", name=f"rsi{k}") for k in range(NQC)
        ]
        rs_out = [
            dram.tile([QCH // GC, C], BF16, tag=f"rso{k}", name=f"rso{k}")
            for k in range(NQC)
        ]

        # ---- LayerNorm stats helpers ----
        # Stats for a batch of token-major [P, C] tiles; ONE batched sqrt so
        # the scalar engine swaps its activation table (EXP<->SQRT) once per
        # chunk instead of once per tile.
        nsub = C // 512

        def ln_stats_batch(x_ts, tagp):
            n = len(x_ts)
            mv = stat.tile([P, n, 2], FP32, tag=f"{tagp}_mv", name=f"{tagp}_mv")
            nm = stat.tile([P, n], FP32, tag=f"{tagp}_nm", name=f"{tagp}_nm")
            for i, x_t in enumerate(x_ts):
                stats = stat.tile(
                    [P, nsub, 6], FP32, tag=f"{tagp}_st", name=f"{tagp}_st"
                )
                xr = x_t.rearrange("p (n f) -> p n f", n=nsub)
                for s in range(nsub):
                    nc.vector.bn_stats(out=stats[:, s, :], in_=xr[:, s, :])
                nc.vector.bn_aggr(out=mv[:, i, :], in_=stats)
            # mv[:,:,0]=mean  mv[:,:,1]=var -> rstd in place (batched sqrt)
            nc.scalar.activation(
                out=mv[:, :, 1], in_=mv[:, :, 1], func=AF.Sqrt, bias=eps_t, scale=1.0
            )
            nc.vector.reciprocal(out=mv[:, :, 1], in_=mv[:, :, 1])
            for i in range(n):
                nc.vector.tensor_scalar(
                    out=nm[:, i : i + 1],
                    in0=mv[:, i, 0:1],
                    scalar1=-1.0,
                    scalar2=mv[:, i, 1:2],
                    op0=ALU.mult,
                    op1=ALU.mult,
                )
            return mv, nm

        def copy_scaled(dst, src, g_ap, b_ap, on_vector):
            if on_vector:
                nc.vector.tensor_scalar(
                    out=dst, in0=src, scalar1=g_ap, scalar2=b_ap,
                    op0=ALU.mult, op1=ALU.add,
                )
            else:
                nc.scalar.activation(
                    out=dst, in_=src, func=AF.Identity, bias=b_ap, scale=g_ap
                )

        # ================= per-chunk emitters =================

        def emit_A(ch):
            """LN1 + transpose + V for token tiles of chunk ch.

            Transposes run in bf16 (fp32 PE transposes are 2-pass, ~4x the
            cost), and the V contraction interleaves token-tile pairs so
            consecutive matmuls accumulate into different PSUM banks.
            """
            its = list(range(ch * KPC, (ch + 1) * KPC))
            x_ts = {}
            for it in its:
                if ch == 0:
                    x_ts[it] = x0_ts[it]  # preloaded before the weight bulk
                    continue
                x_t = xtp.tile([P, C], FP32, tag="xb", name="xt")
                nc.sync.dma_start(x_t, x_full[it * P : (it + 1) * P, :])
                x_ts[it] = x_t
            mv, nm = ln_stats_batch([x_ts[it] for it in its], "ln1")
            for i, it in enumerate(its):
                h_t = tok.tile([P, C], BF16, tag="tb", name="ht")
                # h = (x - mu) * rstd  == x*rstd + (-mu*rstd)
                nc.vector.tensor_scalar(
                    out=h_t, in0=x_ts[it], scalar1=mv[:, i, 1:2], scalar2=nm[:, i : i + 1],
                    op0=ALU.mult, op1=ALU.add,
                )
                for ic in range(NCc):
                    ps = psQ.tile([P, P], BF16, tag="psQ", name="psQ")
                    nc.tensor.transpose(ps, h_t[:, ic * P : (ic + 1) * P], ident_bf)
                    copy_scaled(
                        hT[ic][:, it * P : (it + 1) * P],
                        ps,
                        g1_sb[:, ic : ic + 1],
                        be1_sb[:, ic : ic + 1],
                        on_vector=((it + ic) % 2 == 0),
                    )
            # V (token-major [P, DL]), token tiles pairwise interleaved
            for it0 in its[::2]:
                pair = (it0, it0 + 1)
                pss = {}
                for it in pair:
                    pss[it] = psQ.tile([P, DL], FP32, tag="psQ", name="psQ")
                for ic in range(NCc):
                    for it in pair:
                        nc.tensor.matmul(
                            pss[it],
                            lhsT=hT[ic][:, it * P : (it + 1) * P],
                            rhs=wv_sb[:, ic, :],
                            start=(ic == 0),
                            stop=(ic == NCc - 1),
                        )
                for it in pair:
                    # [P, head, 65]: col 64 = ones so the AV matmul also
                    # produces the softmax rowsum (no separate rowsum matmul)
                    v_t = vp.tile([P, NHL, HD + 1], BF16, tag="v", name="v")
                    if it % 2 == 0:
                        nc.vector.tensor_copy(
                            v_t[:, :, 0:HD],
                            pss[it].rearrange("p (h d) -> p h d", h=NHL),
                        )
                    else:
                        nc.scalar.copy(
                            v_t[:, :, 0:HD],
                            pss[it].rearrange("p (h d) -> p h d", h=NHL),
                        )
                    nc.vector.memset(v_t[:, :, HD], 1.0)
                    V4[it] = v_t

        def emit_B(ch):
            """Q^T / K^T (feature-major, head-pair stacked) for chunk ch.

            Q and K contractions interleave so consecutive matmuls hit
            different PSUM banks (keeps the PE fill/drain pipelined).
            """
            for pr in range(NPAIR):
                ps_q = psQ.tile([P, QCH], FP32, tag="psQ", name="psQ")
                ps_k = psQ.tile([P, QCH], FP32, tag="psQ", name="psQ")
                for ic in range(NCc):
                    for ps, w_sb in ((ps_q, wq_sb), (ps_k, wk_sb)):
                        nc.tensor.matmul(
                            ps,
                            lhsT=w_sb[:, ic, pr * P : (pr + 1) * P],
                            rhs=hT[ic][:, ch * QCH : (ch + 1) * QCH],
                            start=(ic == 0),
                            stop=(ic == NCc - 1),
                        )
                for qi, (dst_tiles, ps) in enumerate(((QT, ps_q), (KT, ps_k))):
                    dst = dst_tiles[pr][:, ch * QCH : (ch + 1) * QCH]
                    if (pr + qi) % 2 == 0:
                        nc.vector.tensor_copy(dst, ps)
                    else:
                        nc.scalar.copy(dst, ps)

        def emit_C(ch):
            """scores + softmax-exp + AV + normalize + proj + ReduceScatter."""
            n_kt = (ch + 1) * KPC
            attT = []
            for pr in range(NPAIR):
                att_ps = [
                    psAV.tile([P, QCH], FP32, tag="psAV", name="psAV")
                    for _ in range(2)
                ]
                # scores tile per k-tile holds BOTH heads ((pos, q) layout):
                # ONE exp instruction unlocks both heads' AVs, and the psS
                # double-buffer gives a true depth-2 pipeline (exp of k-tile
                # n overlaps the AVs of n-1 and scores of n+1).
                for kt in range(n_kt):
                    s_ps = psS.tile([P, 2, QCH], FP32, tag="psS", name="psS")
                    for pos in range(2):
                        nc.tensor.matmul(
                            s_ps[:, pos, :],
                            lhsT=KT[pr][
                                64 * pos : 64 * pos + 64, kt * P : (kt + 1) * P
                            ],
                            rhs=QT[pr][
                                64 * pos : 64 * pos + 64,
                                ch * QCH : (ch + 1) * QCH,
                            ],
                            start=True,
                            stop=True,
                            tile_position=(64 * pos, 0),
                        )
                    e_sb = exps.tile([P, 2, QCH], BF16, tag="e", name="e")
                    nc.scalar.activation(
                        out=e_sb, in_=s_ps, func=AF.Exp, scale=scale
                    )
                    jd = kt - ch * KPC
                    if 0 <= jd < KPC:
                        # cols < jd*P fully masked; [jd*P,(jd+1)*P)
                        # triangular; rest fully visible.
                        for pos in range(2):
                            if jd > 0:
                                nc.vector.memset(e_sb[:, pos, 0 : jd * P], 0.0)
                            nc.vector.tensor_mul(
                                e_sb[:, pos, jd * P : (jd + 1) * P],
                                e_sb[:, pos, jd * P : (jd + 1) * P],
                                mask_sb,
                            )
                    # AV (rowsum rides along as the 65th lhsT column);
                    # pos streams alternate PSUM banks
                    for pos in range(2):
                        lh = 2 * pr + pos
                        nc.tensor.matmul(
                            att_ps[pos][0:65, :],
                            lhsT=V4[kt][:, lh, :],
                            rhs=e_sb[:, pos, :],
                            start=(kt == 0),
                            stop=(kt == n_kt - 1),
                        )
                # normalize: reciprocal on the [1, QCH] rowsum row, broadcast
                # via outer-product matmul, multiply on vector.
                bc_ps = psQ.tile([P, QCH], FP32, tag="psQ", name="psQ")
                for pos in range(2):
                    rrec = stat.tile([1, QCH], BF16, tag=f"rr{pos}", name="rrec")
                    with nc.allow_low_precision(reason="1/rowsum as bf16 matmul rhs"):
                        nc.vector.reciprocal(out=rrec, in_=att_ps[pos][64:65, :])
                    nc.tensor.matmul(
                        bc_ps[64 * pos : 64 * pos + 64, :],
                        lhsT=ones_t,
                        rhs=rrec,
                        start=True,
                        stop=True,
                        tile_position=(0, 64 * pos),
                        skip_group_check=(pos == 1),
                    )
                rec_sb = exps.tile([P, QCH], BF16, tag="rsb", name="rsb")
                nc.scalar.copy(rec_sb, bc_ps)
                at = attp.tile([P, QCH], BF16, tag="attT", name="attT")
                for pos in range(2):
                    rows = slice(64 * pos, 64 * pos + 64)
                    nc.vector.tensor_mul(
                        at[rows, :], att_ps[pos][0:64, :], rec_sb[rows, :]
                    )
                attT.append(at)
            # proj partials (local heads, all chunk tokens) -> bf16 -> rs_in;
            # the two output-column halves interleave (PSUM bank alternation)
            for tt in range(KPC):
                pjs = [psQ.tile([P, 512], FP32, tag="psQ", name="psQ") for _ in range(2)]
                for pr in range(NPAIR):
                    for oc in range(2):
                        nc.tensor.matmul(
                            pjs[oc],
                            lhsT=attT[pr][:, tt * P : (tt + 1) * P],
                            rhs=wp_sb[:, pr, oc * 512 : (oc + 1) * 512],
                            start=(pr == 0),
                            stop=(pr == NPAIR - 1),
                        )
                for oc in range(2):
                    pj_sb = exps.tile([P, 512], BF16, tag="pj", name="pjsb")
                    if (tt + oc) % 2 == 0:
                        nc.vector.tensor_copy(pj_sb, pjs[oc])
                    else:
                        nc.scalar.copy(pj_sb, pjs[oc])
                    nc.sync.dma_start(
                        rs_in[ch][tt * P : (tt + 1) * P, oc * 512 : (oc + 1) * 512],
                        pj_sb,
                    )
            nc.gpsimd.collective_compute(
                "ReduceScatter",
                ALU.add,
                replica_groups=groups,
                ins=[rs_in[ch][:].opt()],
                outs=[rs_out[ch][:].opt()],
            )

        def emit_P(ch):
            """residual for our 128-token block of chunk ch + LN2."""
            st = ch  # shard token tile == chunk index
            r_t = atgp.tile([P, C], BF16, tag="atg", name="rt")
            nc.sync.dma_start(r_t, rs_out[ch][:])
            xs_t = tokE.tile([P, C], FP32, tag="te", name="xst")
            nc.sync.dma_start(xs_t, x_shard[st * P : (st + 1) * P, :])
            x2 = x2pb[st]
            nc.vector.tensor_add(x2, r_t, xs_t)
            nc.gpsimd.tensor_add(x2, x2, bp_bc)
            mv, nm = ln_stats_batch([x2], "ln2")
            h2f = tokE.tile([P, C], BF16, tag="te", name="h2f")
            nc.vector.tensor_scalar(
                out=h2f, in0=x2, scalar1=mv[:, 0, 1:2], scalar2=nm[:, 0:1],
                op0=ALU.mult, op1=ALU.add,
            )
            for ic in range(NCc):
                ps = psQ.tile([P, P], BF16, tag="psQ", name="psQ")
                nc.tensor.transpose(ps, h2f[:, ic * P : (ic + 1) * P], ident_bf)
                copy_scaled(
                    h2T[:, ic, st * P : (st + 1) * P],
                    ps,
                    g2_sb[:, ic : ic + 1],
                    be2_sb[:, ic : ic + 1],
                    on_vector=((st + ic) % 2 == 0),
                )
            # after LN2 consumed x2, fold in b2 for the FFN residual
            nc.gpsimd.tensor_add(x2, x2, b2_bc)

        # ================= emission =================
        emit_A(0)
        emit_B(0)
        for ch in range(NQC):
            emit_C(ch)
            if ch + 1 < NQC:
                emit_A(ch + 1)
                emit_B(ch + 1)
        # all residual+LN2 blocks after the attention stream: P(0..2) fill
        # the RS3 tail window, and no P op can head-of-line-block the
        # engine queues mid-attention.
        for ch in range(NQC):
            emit_P(ch)

        # ================= FFN =================
        HPC = (2 * T) // TSH  # h-tiles per hidT chunk (8)
        n_hc = (NHT + HPC - 1) // HPC
        hidT = [
            hid.tile([P, 2 * T], BF16, tag="hid", name=f"hid{j}") for j in range(n_hc)
        ]

        def hid_slice(ht, t0, tlen):
            j, o = ht // HPC, ht % HPC
            return hidT[j][:, o * TSH + t0 : o * TSH + t0 + tlen]

        n_grp = (NST + 1) // 2
        grp0 = [
            psS.tile([P, min(2, NST) * 512], FP32, tag="psS", name="psS")
            for _ in range(n_grp)
        ]

        def grp_slice(grp, tt):
            return grp[tt // 2][:, (tt % 2) * 512 : (tt % 2 + 1) * 512]

        def emit_w2(ht, w2t, grp):
            for tt in range(NST):
                nc.tensor.matmul(
                    grp_slice(grp, tt),
                    lhsT=hid_slice(ht, tt * P, P),
                    rhs=w2t,
                    start=(ht == 0),
                    stop=(ht == NHT - 1),
                )

        pending = []  # W2 of a tile pair trails the next pair's hid matmuls
        # so the relu is never on the PE critical path.
        for h2 in range(NHT // 2):
            # hid matmuls for a pair of h-tiles, interleaved so consecutive
            # matmuls hit different PSUM banks
            hts = (2 * h2, 2 * h2 + 1)
            w1t = w1p.tile([P, NCc, 2 * P], BF16, tag="w1t", name="w1t")
            for ic in range(NCc):
                nc.sync.dma_start(
                    w1t[:, ic, :],
                    w1_d[ic * P : (ic + 1) * P, hts[0] * P : (hts[0] + 2) * P],
                )
            hps = {ht: psQ.tile([P, TSH], FP32, tag="psQ", name="psQ") for ht in hts}
            for ic in range(NCc):
                for ht in hts:
                    nc.tensor.matmul(
                        hps[ht],
                        lhsT=w1t[:, ic, (ht % 2) * P : (ht % 2 + 1) * P],
                        rhs=h2T[:, ic, :],
                        start=(ic == 0),
                        stop=(ic == NCc - 1),
                    )
            for ht in hts:
                nc.scalar.activation(
                    out=hid_slice(ht, 0, TSH),
                    in_=hps[ht],
                    func=AF.Relu,
                    bias=b1_sb[:, ht : ht + 1],
                    scale=1.0,
                )
            w2t = w2p.tile([P, 2, 512], BF16, tag="w2t", name="w2t")
            nc.sync.dma_start(
                w2t,
                w2_d[hts[0] * P : (hts[0] + 2) * P, 0:512].rearrange(
                    "(n p) m -> p n m", p=P
                ),
            )
            for ht0, w2t0 in pending:
                for k in range(2):
                    emit_w2(ht0 + k, w2t0[:, k, :], grp0)
            pending = [(hts[0], w2t)]
        for ht0, w2t0 in pending:
            for k in range(2):
                emit_w2(ht0 + k, w2t0[:, k, :], grp0)
        pending = []
        for tt in range(NST):
            ob = obp.tile([P, 512], FP32, tag="ob", name="ob")
            nc.vector.tensor_add(ob, grp_slice(grp0, tt), x2pb[tt][:, 0:512])
            nc.sync.dma_start(out_d[tt * P : (tt + 1) * P, 0:512], ob)
        grp1 = [
            psS.tile([P, min(2, NST) * 512], FP32, tag="psS", name="psS")
            for _ in range(n_grp)
        ]
        for h2 in range(NHT // 2):
            w2t = w2p.tile([P, 2, 512], BF16, tag="w2t", name="w2t")
            nc.sync.dma_start(
                w2t,
                w2_d[2 * h2 * P : (2 * h2 + 2) * P, 512:1024].rearrange(
                    "(n p) m -> p n m", p=P
                ),
            )
            for k in range(2):
                emit_w2(2 * h2 + k, w2t[:, k, :], grp1)
        for tt in range(NST):
            ob = obp.tile([P, 512], FP32, tag="ob", name="ob")
            nc.vector.tensor_add(ob, grp_slice(grp1, tt), x2pb[tt][:, 512:1024])
            nc.sync.dma_start(out_d[tt * P : (tt + 1) * P, 512:1024], ob)

    nc.finalize()
    return nc


# ------------------------- host side -------------------------

_CACHE = {}
LAST_RESULTS = None


def make_in_maps(inputs, T=2048, C=1024, H=16, F=4096, GC=4, n_cores=8):
    HD = 64
    NHL = H // GC
    DL = NHL * HD
    NHT = F // P
    NCc = C // P
    bf = ml_dtypes.bfloat16

    x = np.asarray(inputs["x"], np.float32)
    Wq = np.asarray(inputs["Wq"], np.float32)
    Wk = np.asarray(inputs["Wk"], np.float32)
    Wv = np.asarray(inputs["Wv"], np.float32)
    Wp = np.asarray(inputs["Wp"], np.float32)
    bp = np.asarray(inputs["bp"], np.float32)
    W1 = np.asarray(inputs["W1"], np.float32)
    b1 = np.asarray(inputs["b1"], np.float32)
    W2 = np.asarray(inputs["W2"], np.float32)
    b2 = np.asarray(inputs["b2"], np.float32)
    g1 = np.asarray(inputs["g1"], np.float32)
    be1 = np.asarray(inputs["beta1"], np.float32)
    g2 = np.asarray(inputs["g2"], np.float32)
    be2 = np.asarray(inputs["beta2"], np.float32)

    maskr = np.triu(np.ones((P, P), np.float32)).astype(bf)  # m[kr,qr]=kr<=qr
    b1r = np.ascontiguousarray(b1.reshape(NHT, P).T)
    g1r = np.ascontiguousarray(g1.reshape(NCc, P).T)
    be1r = np.ascontiguousarray(be1.reshape(NCc, P).T)
    g2r = np.ascontiguousarray(g2.reshape(NCc, P).T)
    be2r = np.ascontiguousarray(be2.reshape(NCc, P).T)
    w1b = W1.astype(bf)
    w2b = W2.astype(bf)

    NQC = T // QCH
    RPC = QCH // GC  # shard rows per q-chunk (128)

    def shard_rows(g):
        return np.concatenate(
            [np.arange(k * QCH + g * RPC, k * QCH + (g + 1) * RPC) for k in range(NQC)]
        )

    in_maps = []
    for c in range(n_cores):
        b, g = c // GC, c % GC
        hsl = slice(g * NHL, (g + 1) * NHL)
        in_maps.append(
            {
                "x_full": np.ascontiguousarray(x[b]),
                "x_shard": np.ascontiguousarray(x[b][shard_rows(g)]),
                "wq": np.ascontiguousarray(
                    Wq[hsl].transpose(1, 0, 2).reshape(C, DL)
                ).astype(bf),
                "wk": np.ascontiguousarray(
                    Wk[hsl].transpose(1, 0, 2).reshape(C, DL)
                ).astype(bf),
                "wv": np.ascontiguousarray(
                    Wv[hsl].transpose(1, 0, 2).reshape(C, DL)
                ).astype(bf),
                "wp": np.ascontiguousarray(Wp[g * DL : (g + 1) * DL]).astype(bf),
                "w1": w1b,
                "w2": w2b,
                "b1r": b1r,
                "bp": bp.astype(bf),
                "b2": b2.astype(bf),
                "g1r": g1r,
                "be1r": be1r,
                "g2r": g2r,
                "be2r": be2r,
                "maskr": maskr,
            }
        )
    return in_maps


def kernel(**inputs) -> np.ndarray:
    global LAST_RESULTS
    B, T, C = inputs["x"].shape
    H = inputs["Wq"].shape[0]
    F = inputs["W1"].shape[1]
    GC = 4
    n_cores = 8
    key = (T, C, H, F)
    if key not in _CACHE:
        _CACHE[key] = build_block(T=T, C=C, NHL=H // GC, F=F, GC=GC, n_cores=n_cores)
    nc = _CACHE[key]
    in_maps = make_in_maps(inputs, T=T, C=C, H=H, F=F, GC=GC, n_cores=n_cores)
    res = run_bass_kernel_spmd(nc, in_maps, core_ids=list(range(n_cores)))
    LAST_RESULTS = res
    out = np.empty((B, T, C), np.float32)
    NQC = T // QCH
    RPC = QCH // GC
    for c in range(n_cores):
        b, g = c // GC, c % GC
        sh = res.results[c]["out"]
        for k in range(NQC):
            out[b, k * QCH + g * RPC : k * QCH + (g + 1) * RPC] = sh[
                k * RPC : (k + 1) * RPC
            ]
    return out
